# revision 1
# baseline (speedup 1.0000x reference)
"""Trainium2 Bass kernel for the smoothed Preisach hysteresis model.

Math: the reference per-step update
    s' = where(h_t > h_{t-1}, s + (1-s)*sigmoid((h_t-alpha)/temp),
                              s + (-1-s)*sigmoid((beta-h_t)/temp))
is a first-order linear recurrence s' = (1-g)s + sigma*g. Two changes
of variable make it Trainium-friendly. With u = (s+1)/2 and the
up-mask M_t (1 if h rose, else 0):  u' = a*u + (1-a)*M,
a = sigmoid(-arg). With z = M - u and dM_t = M_t - M_{t-1}:

    z' = (z + dM_t) * a_t

dM is a host-known constant row in {-1,0,1}, and `a` is directly the
ScalarE sigmoid output, so the DVE tensor_tensor_scan (op0=add,
op1=mult, fp32 internal state) consumes exactly one ACT pass and one
broadcast constant -- the scan is the only DVE work, and it is the
hardware bottleneck (~2.4 ns/element, serial recurrence).

arg[n,t] = p_t + alpha_n*q_t + beta_n*r_t is rank-3 in (n,t), computed
by a K=6 fp16 PE matmul with alpha/beta/p split into hi+lo fp16 pairs
(fp32-class accuracy, fp32 PSUM accumulate).

Readout: m_num_t = sum_n d_n s_tn = d16sum*(2*M_t - 1) - 2*sum_n d_n z_tn;
the K=128 fp16 readout matmul accumulates d^T z over the 6 hysteron
tiles; the host applies the affine fixup and epilogue.

Sharding: hysteron dim N=5151 split across 8 cores (644 each, padded
to 6 tiles of 128 partitions; padding carries density 0). Each core
outputs its readout partials [1, T]; host reduces across cores.
"""

import sys

import numpy as np

sys.path.insert(0, "/opt/trn_rl_repo")

N = 5151
T = 4096
TEMP = 0.01
NCORES = 8
P = 128
TILES = 6                 # ceil(644/128); per-core rows padded to 768
NPC = 644                 # hysterons per core (8*644 = 5152 >= N)
ROWS = TILES * P          # 768
K6 = 6                    # arg matmul contraction: a_hi,a_lo,b_hi,b_lo,1,1
CH = 512                  # matmul chunk along T (one PSUM bank fp32)
NCH = T // CH
ACH = 1024                # ACT chunk (2 PSUM banks per arg tile)
SCH = 2048                # scan chunk along T (TTS chained via initial)
NSCH = T // SCH

_PROG_CACHE = {}


def _build_program(reps=1, loop_n=0, skip=()):
    import contextlib

    import concourse.bass as bass
    import concourse.tile as tile
    from concourse import bacc, mybir

    f32 = mybir.dt.float32
    f16 = mybir.dt.float16
    nc = bacc.Bacc("TRN2", target_bir_lowering=False, debug=False,
                   num_devices=NCORES)

    wt_d = nc.dram_tensor("wt", [K6, ROWS], f16, kind="ExternalInput")
    v_d = nc.dram_tensor("v", [K6, T], f16, kind="ExternalInput")
    dm_d = nc.dram_tensor("dm", [T], f16, kind="ExternalInput")
    dens_d = nc.dram_tensor("dens", [P, TILES], f16, kind="ExternalInput")
    mpart_d = nc.dram_tensor("mpart", [1, T], f32, kind="ExternalOutput")

    wt_ap = wt_d.ap()
    v_ap = v_d.ap()
    dm_ap = dm_d.ap()
    dens_ap = dens_d.ap()
    mpart_ap = mpart_d.ap()

    ts = bass.ts
    Sigmoid = mybir.ActivationFunctionType.Sigmoid
    mult = mybir.AluOpType.mult
    add = mybir.AluOpType.add

    with tile.TileContext(nc) as tc:
        from contextlib import ExitStack
        with ExitStack() as ctx:
            consts = ctx.enter_context(tc.tile_pool(name="consts", bufs=1))
            apool = ctx.enter_context(tc.tile_pool(name="a", bufs=6))
            spool = ctx.enter_context(tc.tile_pool(name="s", bufs=TILES))
            mpool = ctx.enter_context(tc.tile_pool(name="m", bufs=1))
            ps_arg = ctx.enter_context(
                tc.tile_pool(name="ps_arg", bufs=3, space="PSUM"))
            ps_m = ctx.enter_context(
                tc.tile_pool(name="ps_m", bufs=2, space="PSUM"))

            wt_sb = consts.tile([K6, ROWS], f16)
            v_sb = consts.tile([K6, T], f16)
            dens_sb = consts.tile([P, TILES], f16)
            dm_bc = consts.tile([P, T], f16)

            nc.sync.dma_start(out=wt_sb[:], in_=wt_ap[:, :])
            nc.sync.dma_start(out=v_sb[:], in_=v_ap[:, :])
            # broadcast dM row to all 128 partitions via 0-stride DMA,
            # chunked across queues so it doesn't serialize the pipeline
            for j in range(NCH):
                src = bass.AP(tensor=dm_ap.tensor,
                              offset=dm_ap.offset + j * CH,
                              ap=[[0, P], [1, CH]])
                nc.sync.dma_start(out=dm_bc[:, ts(j, CH)], in_=src)
            nc.sync.dma_start(out=dens_sb[:], in_=dens_ap[:, :])

            if loop_n:
                loop_cm = tc.For_i(
                    0, loop_n, 1,
                    hint_engines=(mybir.EngineType.PE,
                                  mybir.EngineType.Activation,
                                  mybir.EngineType.DVE))
            else:
                loop_cm = contextlib.nullcontext()
            with loop_cm:
              for _rep in range(reps):
                s_tiles = []
                for i in range(TILES):
                    s = spool.tile([P, T], f16)
                    for c in range(NSCH):
                        a = apool.tile([P, SCH], f16)
                        for aj in range(SCH // ACH):
                            arg = ps_arg.tile([P, ACH], f32, tag="arg")
                            for jj in range(ACH // CH):
                                j = (c * SCH + aj * ACH) // CH + jj
                                nc.tensor.matmul(
                                    out=arg[:, ts(jj, CH)],
                                    lhsT=wt_sb[:, ts(i, P)],
                                    rhs=v_sb[:, ts(j, CH)],
                                    start=True, stop=True,
                                )
                            # a = sigmoid(-arg)
                            nc.scalar.activation(
                                out=a[:, ts(aj, ACH)], in_=arg[:],
                                func=Sigmoid, scale=-1.0)
                        if "scan" not in skip:
                            init = (0.0 if c == 0
                                    else s[:, c * SCH - 1:c * SCH])
                            # z' = (z + dM) * a
                            nc.vector.tensor_tensor_scan(
                                out=s[:, ts(c, SCH)],
                                data0=dm_bc[:, ts(c, SCH)],
                                data1=a[:],
                                initial=init, op0=add, op1=mult,
                            )
                        else:
                            nc.vector.tensor_copy(out=s[:, ts(c, SCH)],
                                                  in_=a[:])
                    s_tiles.append(s)

                m_sb = mpool.tile([1, T], f32)
                for j in range(NCH):
                    mp = ps_m.tile([1, CH], f32)
                    for i in range(TILES):
                        nc.tensor.matmul(
                            out=mp[:],
                            lhsT=dens_sb[:, i:i + 1],
                            rhs=s_tiles[i][:, ts(j, CH)],
                            start=(i == 0), stop=(i == TILES - 1),
                        )
                    nc.scalar.copy(out=m_sb[:, ts(j, CH)], in_=mp[:])
                    nc.sync.dma_start(out=mpart_ap[:, ts(j, CH)],
                                      in_=m_sb[:, ts(j, CH)])
    nc.compile()
    return nc


def _split16(x):
    hi = x.astype(np.float16)
    lo = (x - hi.astype(np.float64)).astype(np.float16)
    return hi, lo


def _host_prep(h, mesh_points, raw_density):
    h = np.asarray(h, np.float32)
    mesh = np.asarray(mesh_points, np.float32)
    rd = np.asarray(raw_density, np.float32)
    beta = mesh[:, 0].astype(np.float64)
    alpha = mesh[:, 1].astype(np.float64)

    hprev = np.concatenate([[np.float32(0.0)], h[:-1]])
    up = h > hprev
    R = np.float64(1.0) / np.float64(np.float32(TEMP))
    h64 = h.astype(np.float64)
    q = np.where(up, -R, 0.0)
    r = np.where(up, 0.0, R)
    p = np.where(up, R * h64, -R * h64)
    p_hi, p_lo = _split16(p)
    q16 = q.astype(np.float16)
    r16 = r.astype(np.float16)
    V6 = np.stack([q16, q16, r16, r16, p_hi, p_lo]).astype(np.float16)

    M = up.astype(np.float64)                 # M_t in {0,1}
    Mprev = np.concatenate([[0.0], M[:-1]])
    dM = (M - Mprev).astype(np.float16)       # in {-1,0,1}

    dens = (1.0 / (1.0 + np.exp(-rd.astype(np.float64))))  # [N] float64

    pad = NCORES * NPC - N   # 1
    alpha_p = np.concatenate([alpha, np.full(pad, 0.5)])
    beta_p = np.concatenate([beta, np.full(pad, 0.5)])
    dens_p = np.concatenate([dens, np.zeros(pad)])

    in_maps = []
    d16sum = 0.0
    for c in range(NCORES):
        sl = slice(c * NPC, (c + 1) * NPC)
        a_c = np.full(ROWS, 0.5)
        b_c = np.full(ROWS, 0.5)
        d_c = np.zeros(ROWS)
        a_c[:NPC] = alpha_p[sl]
        b_c[:NPC] = beta_p[sl]
        d_c[:NPC] = dens_p[sl]
        ah, al = _split16(a_c)
        bh, bl = _split16(b_c)
        wt = np.stack([ah, al, bh, bl,
                       np.ones(ROWS, np.float16),
                       np.ones(ROWS, np.float16)]).astype(np.float16)
        dens16 = d_c.astype(np.float16)
        dens_tiles = dens16.reshape(TILES, P).T  # [P, TILES]
        d16sum += dens16.astype(np.float64).sum()
        in_maps.append({
            "wt": wt,
            "v": V6,
            "dm": dM,
            "dens": dens_tiles,
        })
    return in_maps, dens, h, d16sum, M


def kernel(h, mesh_points, raw_density, raw_offset, raw_scale, raw_slope):
    from concourse.bass_utils import run_bass_kernel_spmd

    in_maps, dens, h32, d16sum, M = _host_prep(h, mesh_points, raw_density)

    if "prog" not in _PROG_CACHE:
        _PROG_CACHE["prog"] = _build_program()
    nc = _PROG_CACHE["prog"]

    res = run_bass_kernel_spmd(nc, in_maps, list(range(NCORES)))
    zpart = np.zeros(T, np.float64)
    for c in range(NCORES):
        zpart += res.results[c]["mpart"].astype(np.float64).reshape(T)

    def sigm(x):
        return 1.0 / (1.0 + np.exp(-np.float64(np.asarray(x, np.float32)[0])))

    offset = -10.0 + 20.0 * sigm(raw_offset)
    scale = 20.0 * sigm(raw_scale)
    slope = -20.0 + 40.0 * sigm(raw_slope)

    # s = 2u-1, u = M - z  =>  sum(d*s) = d16sum*(2M-1) - 2*sum(d*z)
    m = (d16sum * (2.0 * M - 1.0) - 2.0 * zpart) / dens.sum()
    out = scale * m + h32.astype(np.float64) * slope + offset
    return out.astype(np.float32)

